# revision 46
# baseline (speedup 1.0000x reference)
"""Trainium2 Bass kernel for nn_B_Conv2d_ConvNN_K_N (retrieval_knn).

Data-parallel over 8 NeuronCores: 32 images/core, weights replicated.

KNN neighbor aggregation is reformulated as a one-hot matmul: per image,
P_k = (W_k @ samp)^T, and a rank-k membership mask A_k[t, n] =
(score[t,n] == kth_max[t]) / count, so y[o,t] = sum_k (P_k^T A_k^T)[o,t].
Count normalization makes duplicated sample columns exact.  This removes
all per-token gathers and the DRAM index staging of the previous version.

Convs use unpadded clipped-tap matmuls (border taps write PSUM subranges).
FC1 keeps activations stationary (LDW [128,32] per position) and streams
the 64MB weight through a deep-prefetch tile ring.
"""
import sys
if '/opt/trn_rl_repo' not in sys.path:
    sys.path.insert(0, '/opt/trn_rl_repo')

import numpy as np
import concourse.bacc as bacc
import concourse.mybir as mybir
from concourse.tile import TileContext
from concourse.bass_utils import run_bass_kernel_spmd

dt = mybir.dt
AF = mybir.ActivationFunctionType
ALU = mybir.AluOpType
NCORES = 8
B = 32            # images per core
T = 256           # tokens per image at conv resolution (16x16)
NT = B * T        # 8192
NS = 64           # random samples per image
K = 9             # nearest neighbors
NFCW = 5          # fc1 weight tiles in flight (8KB free-bytes each)


def _bf16(x):
    import ml_dtypes
    return np.asarray(x, np.float32).astype(ml_dtypes.bfloat16)


def prep_weights(w):
    """Core-independent input tensors (weights, indices)."""
    m = {}

    def samp_idx(idx, groups):
        # image index local to each 8-image gather block
        t = np.zeros((16 * groups, 128), np.int16)
        s = np.arange(128)
        for g in range(groups):
            for p in range(16):
                t[16 * g + p, :] = ((s // 4) % 8) * 256 + idx[16 * (s % 4) + p]
        return t
    m['idxs2'] = samp_idx(np.asarray(w['idx2']), 4)

    w1a = np.asarray(w['w1a'], np.float32)       # (16, 12, 3, 3)
    wc1 = np.zeros((12, 144), np.float32)
    for dh in range(3):
        for dw in range(3):
            tap = dh * 3 + dw
            wc1[:, tap * 16:(tap + 1) * 16] = w1a[:, :, dh, dw].T
    m['wc1'] = wc1
    w1b = np.asarray(w['w1b'], np.float32)       # (16, 12, 9)
    w1bkT = np.zeros((16, 144), np.float32)
    for k in range(K):
        w1bkT[0:12, k * 16:(k + 1) * 16] = w1b[:, :, k].T
    m['w1bkT'] = w1bkT
    w1p = np.asarray(w['w1p'], np.float32)[:, :, 0, 0]   # (64, 32)
    w1pl = np.zeros((48, 64), np.float32)
    w1pl[0:16] = w1p[:, 0:16].T
    w1pl[32:48] = w1p[:, 16:32].T
    m['w1p_l'] = w1pl

    w2a = np.asarray(w['w2a'], np.float32)       # (32, 64, 3, 3)
    wc2 = np.zeros((64, 288), np.float32)
    for dh in range(3):
        for dw in range(3):
            tap = dh * 3 + dw
            wc2[:, tap * 32:(tap + 1) * 32] = w2a[:, :, dh, dw].T
    m['wconv2'] = _bf16(wc2)
    w2b = np.asarray(w['w2b'], np.float32)       # (32, 64, 9)
    w2bkT = np.zeros((64, 288), np.float32)
    for k in range(K):
        w2bkT[:, k * 32:(k + 1) * 32] = w2b[:, :, k].T
    m['w2bkT'] = w2bkT
    w2p = np.asarray(w['w2p'], np.float32)[:, :, 0, 0]   # (128, 64)
    m['w2p_l'] = _bf16(np.ascontiguousarray(w2p.T))      # (64, 128)

    m['b1a'] = np.asarray(w['b1a']).reshape(16, 1).astype(np.float32)
    m['b1b'] = np.asarray(w['b1b']).reshape(16, 1).astype(np.float32)
    m['b1p'] = np.asarray(w['b1p']).reshape(64, 1).astype(np.float32)
    m['b2a'] = np.asarray(w['b2a']).reshape(32, 1).astype(np.float32)
    m['b2b'] = np.asarray(w['b2b']).reshape(32, 1).astype(np.float32)
    m['b2p'] = np.asarray(w['b2p']).reshape(128, 1).astype(np.float32)
    m['onesc2'] = np.ones((64, 1), np.float32)
    m['ones1'] = np.ones((1, 32), np.float32)
    m['neghalf'] = np.full((1, 128), -0.5, np.float32)
    def dup_scale(idx):
        idx = np.asarray(idx)
        mult = np.array([(idx == v).sum() for v in idx], np.float32)
        return np.tile(1.0 / mult, 2).reshape(128, 1)
    m['dup1sc'] = dup_scale(w['idx1'])
    m['dup2sc'] = dup_scale(w['idx2'])
    m['ident128'] = np.eye(128, dtype=np.float32)
    m['ident128b'] = _bf16(np.eye(128, dtype=np.float32))

    # FC1 weights per spatial position, bf16, 4-pos tiles (64, 128, 4096)
    fc1 = np.asarray(w['fc1w']).reshape(1024, 32, 32, 32)   # f, cc, hh, ww
    fc1 = fc1.reshape(1024, 32, 16, 2, 16, 2)               # f, cc, h, i, w, j
    fc1 = fc1.transpose(2, 4, 1, 3, 5, 0)                   # h, w, cc, i, j, f
    fc1 = fc1.reshape(256, 128, 1024)                       # pos, ch, f
    m['fc1wp'] = _bf16(fc1.reshape(64, 4, 128, 1024).transpose(0, 2, 1, 3)
                       .reshape(64, 128, 4096))
    m['fc1b2'] = np.asarray(w['fc1b']).reshape(1, 1024).astype(np.float32)
    m['fc2w_l'] = np.ascontiguousarray(np.asarray(w['fc2w']).T).astype(np.float32)
    m['fc2b'] = np.asarray(w['fc2b']).reshape(10, 1).astype(np.float32)
    return m


def prep_x(x_core, idx1):
    """Per-core tensors: unshuffled x, host-gathered samples, sample norms."""
    xu = x_core.reshape(B, 3, 16, 2, 16, 2).transpose(1, 3, 5, 0, 2, 4)
    xu = np.ascontiguousarray(xu).reshape(12, NT).astype(np.float32)
    xun = np.zeros((16, NT), np.float32)
    xun[:12] = xu
    xun[12] = 1.0
    samp = xu.reshape(12, B, T)[:, :, idx1]        # (12, B, 64)
    samp1 = np.zeros((16, B * NS), np.float32)
    samp1[:12] = samp.reshape(12, B * NS)
    samp1[12] = -0.5 * (samp1[:12] ** 2).sum(axis=0)
    return {'xun': xun, 'samp1': samp1}


def prep_core_maps(inputs):
    x = np.asarray(inputs['x'], np.float32)
    idx1 = np.asarray(inputs['idx1'])
    wmap = prep_weights(inputs)
    maps = []
    for c in range(NCORES):
        m = dict(wmap)
        m.update(prep_x(x[c * B:(c + 1) * B], idx1))
        maps.append(m)
    return maps


def build_bass(stage=3):
    F32, I16, BF16 = dt.float32, dt.int16, dt.bfloat16
    nc = bacc.Bacc("TRN2", target_bir_lowering=False, debug=False)

    def din(name, shape, d=F32):
        return nc.dram_tensor(name, shape, d, kind="ExternalInput")

    xun_d = din('xun', [16, NT])
    samp1_d = din('samp1', [16, 2048])
    idxs2_d = din('idxs2', [64, 128], I16)
    wc1_d = din('wc1', [12, 144])
    w1bkT_d = din('w1bkT', [16, 144])
    w1p_l_d = din('w1p_l', [48, 64])
    wconv2_d = din('wconv2', [64, 288], BF16)
    w2bkT_d = din('w2bkT', [64, 288])
    w2p_l_d = din('w2p_l', [64, 128], BF16)
    b1a_d, b1b_d, b1p_d = din('b1a', [16, 1]), din('b1b', [16, 1]), din('b1p', [64, 1])
    b2a_d, b2b_d, b2p_d = din('b2a', [32, 1]), din('b2b', [32, 1]), din('b2p', [128, 1])
    onesc2_d = din('onesc2', [64, 1])
    ones1_d = din('ones1', [1, 32])
    dup1sc_d = din('dup1sc', [128, 1])
    dup2sc_d = din('dup2sc', [128, 1])
    ident128_d = din('ident128', [128, 128])
    ident128b_d = din('ident128b', [128, 128], BF16)
    fc1wp_d = din('fc1wp', [64, 128, 4096], BF16)
    fc1b2_d = din('fc1b2', [1, 1024])
    fc2w_l_d = din('fc2w_l', [1024, 10])
    fc2b_d = din('fc2b', [10, 1])
    out_d = nc.dram_tensor('out', [B, 10], F32, kind="ExternalOutput")
    if stage == 1:
        h2_d = nc.dram_tensor('h2_dbg', [64, NT], F32, kind="ExternalOutput")
        yc1_d = nc.dram_tensor('yc1_dbg', [48, NT], F32, kind="ExternalOutput")
        ndtb_d = nc.dram_tensor('ndtb_dbg', [128, 4096], F32, kind="ExternalOutput")
        pt1_d = nc.dram_tensor('pt1_dbg', [128, B * 5 * 16], F32, kind="ExternalOutput")
        at0_d = nc.dram_tensor('at0_dbg', [128, 5 * 256], F32, kind="ExternalOutput")
    if stage == 2:
        yp2_d = nc.dram_tensor('yp2_dbg', [128, NT], BF16, kind="ExternalOutput")

    with TileContext(nc) as tc:
        with tc.tile_pool(name="consts", bufs=1) as cpool, \
             tc.tile_pool(name="big", bufs=1) as big, \
             tc.tile_pool(name="att", bufs=2) as att, \
             tc.tile_pool(name="work", bufs=3) as work, \
             tc.tile_pool(name="apool", bufs=4) as apool, \
             tc.tile_pool(name="small", bufs=1) as small, \
             tc.tile_pool(name="fcw", bufs=NFCW) as fcw:

            def load(dram_t, name, d=None):
                t = cpool.tile(list(dram_t.shape), d or dram_t.dtype, name=name)
                nc.scalar.dma_start(t[:], dram_t[:])
                return t

            wc1 = load(wc1_d, 'wc1')
            w1bkT = load(w1bkT_d, 'w1bkT')
            w1p_l = load(w1p_l_d, 'w1p_l')
            wconv2 = load(wconv2_d, 'wconv2')
            w2bkT = load(w2bkT_d, 'w2bkT')
            w2p_l = load(w2p_l_d, 'w2p_l')
            b1a, b1b, b1p = load(b1a_d, 'b1a'), load(b1b_d, 'b1b'), load(b1p_d, 'b1p')
            b2a, b2b, b2p = load(b2a_d, 'b2a'), load(b2b_d, 'b2b'), load(b2p_d, 'b2p')
            onesc2 = load(onesc2_d, 'onesc2')
            ones1 = load(ones1_d, 'ones1')
            dup1sc = load(dup1sc_d, 'dup1sc')
            dup2sc = load(dup2sc_d, 'dup2sc')
            ident128 = load(ident128_d, 'ident128')
            ident128b = load(ident128b_d, 'ident128b')
            fc1b2 = load(fc1b2_d, 'fc1b2')
            fc2b = load(fc2b_d, 'fc2b')
            idxs2 = load(idxs2_d, 'idxs2')
            samp1 = small.tile([16, 2048], F32, tag="samp", name='samp1')
            nc.scalar.dma_start(samp1[:], samp1_d[:])
            fc2w = cpool.tile([128, 80], F32, name='fc2w')
            for r in range(8):
                nc.scalar.dma_start(fc2w[:, r * 10:(r + 1) * 10],
                                    fc2w_l_d[r * 128:(r + 1) * 128, :])

            xun = big.tile([16, NT], F32, tag="act", name='xun')
            nc.sync.dma_start(xun[:], xun_d[:])

            # FC1 weight stream (program-order early; consumed at the end)
            fc1w_tiles = []
            for i in range(64):
                ft = fcw.tile([128, 4096], BF16, tag="fc1w", name=f'fc1w{i}')
                eng = (nc.sync, nc.scalar)[i % 2]
                eng.dma_start(ft[:], fc1wp_d[i])
                fc1w_tiles.append(ft)

            # Per-image P tiles: rows kn (k*64+n) in 5 chunks of 128, cols o
            Pt1 = cpool.tile([128, B, 5, 16], F32, name='Pt1')
            Pt2 = cpool.tile([128, B, 5, 32], BF16, name='Pt2')

            with tc.tile_pool(name="ps", bufs=2, space="PSUM") as ps:

                def conv_loop(lay, o_ch, wtap, cin, srcv, ycat, biasc):
                    taps = [(1, 1)] + [(dh, dw) for dh in range(3)
                                       for dw in range(3)
                                       if not (dh == 1 and dw == 1)]
                    for ch in range(16):
                        b0 = ch * 2
                        pc = ps.tile([o_ch, 2, 16, 16], F32, tag="pc",
                                     name=f'c{lay}_{ch}')
                        for i, (dh, dw) in enumerate(taps):
                            tap = dh * 3 + dw
                            hl, hh = max(0, 1 - dh), min(16, 17 - dh)
                            wl, wh = max(0, 1 - dw), min(16, 17 - dw)
                            nc.tensor.matmul(
                                pc[:, :, hl:hh, wl:wh],
                                wtap[:, tap * o_ch:(tap + 1) * o_ch],
                                srcv[0:cin, b0:b0 + 2,
                                     hl + dh - 1:hh + dh - 1,
                                     wl + dw - 1:wh + dw - 1],
                                start=(i == 0), stop=(i == 8))
                        nc.scalar.activation(
                            ycat[0:o_ch, ch * 512:(ch + 1) * 512],
                            pc[:].rearrange("o b h w -> o (b h w)"),
                            AF.Relu, bias=biasc[:])

                def score_loop(lay, srcT, ctr_s, samp, ndtb):
                    # row ctr_s-1 of srcT is 1.0 and of samp is -sn/2, so a
                    # single matmul yields dots - sn/2
                    for i in range(B):
                        pd = ps.tile([128, 128], F32, tag="pd",
                                     name=f'd{lay}_{i}')
                        for h in range(2):
                            bh = 2 * i + h
                            nc.tensor.matmul(
                                pd[:, h * NS:(h + 1) * NS],
                                srcT[0:ctr_s, bh * 128:(bh + 1) * 128],
                                samp[0:ctr_s, i * NS:(i + 1) * NS],
                                start=True, stop=True)
                        nc.scalar.copy(ndtb[:, i * 128:(i + 1) * 128], pd[:])

                def p_loop(lay, samp, ctr_p, wbkT, o_ch, Pt, dupsc):
                    for b in range(B):
                        pP = ps.tile([128, 5, o_ch], F32, tag="pP",
                                     name=f'pP{lay}_{b}')
                        for c in range(5):
                            for kk in range(1 if c == 4 else 2):
                                k = 2 * c + kk
                                nc.tensor.matmul(
                                    pP[64 * kk:64 * kk + 64, c, :],
                                    samp[0:ctr_p, b * NS:(b + 1) * NS],
                                    wbkT[0:ctr_p, k * o_ch:(k + 1) * o_ch],
                                    start=True, stop=True)
                        nc.scalar.mul(Pt[:, b, :, :], pP[:], dupsc[:])

                def mask_loop(lay, ndtb, o_ch, Pt, ycat, biasb, adt,
                              post_main=None):
                    """Rank one-hot masks -> transposed one-hot matmul."""
                    for b in range(B):
                        AT = att.tile([128, 5, 256], adt, tag="AT",
                                      name=f'AT{lay}_{b}')
                        As = []
                        for h in range(2):
                            bh = b * 2 + h
                            ndt = ndtb[:, bh * NS:(bh + 1) * NS]
                            mxc = work.tile([128, 9], F32, tag="mxc",
                                            name=f'mxc{lay}_{bh}')
                            nc.vector.max(mxc[:, 0:8], ndt)
                            nd2 = work.tile([128, NS], F32, tag="nd2",
                                            name=f'n2_{lay}_{bh}')
                            nc.vector.match_replace(nd2[:], mxc[:, 0:8], ndt,
                                                    -1e30)
                            t9 = work.tile([128, 8], F32, tag="t9",
                                           name=f't9_{lay}_{bh}')
                            nc.vector.max(t9[:], nd2[:])
                            nc.vector.tensor_copy(mxc[:, 8:9], t9[:, 0:1])
                            A = apool.tile([128, 576], adt, tag="A",
                                           name=f'A{lay}_{bh}')
                            nc.vector.tensor_tensor(
                                A[:].rearrange("p (k n) -> p k n", n=NS),
                                ndt.unsqueeze(1).broadcast_to([128, K, NS]),
                                mxc[:].unsqueeze(2).broadcast_to([128, K, NS]),
                                ALU.is_equal)
                            As.append(A)
                        # transpose both halves of chunk c into one PSUM tile,
                        # evacuate with a single scalar copy
                        for c in range(5):
                            W = 64 if c == 4 else 128
                            ptr = ps.tile([128, 256], adt, tag="ptr",
                                          name=f'tr{lay}_{b}_{c}')
                            ident = ident128 if adt == F32 else ident128b
                            for h in range(2):
                                nc.tensor.transpose(
                                    ptr[0:W, h * 128:(h + 1) * 128],
                                    As[h][:, c * 128:c * 128 + W],
                                    ident[:])
                            nc.scalar.copy(AT[0:W, c, :], ptr[0:W, :])
                        if stage == 1 and lay == 1 and b == 0:
                            nc.sync.dma_start(at0_d[:], AT[:])
                        pm = ps.tile([o_ch, 256], F32, tag="pd",
                                     name=f'pm{lay}_{b}')
                        for c in range(5):
                            W = 64 if c == 4 else 128
                            nc.tensor.matmul(pm[:], Pt[0:W, b, c, :],
                                             AT[0:W, c, :],
                                             start=(c == 0), stop=(c == 4))
                        nc.scalar.activation(
                            ycat[32:32 + o_ch, b * T:(b + 1) * T], pm[:],
                            AF.Relu, bias=biasb[:])
                        if post_main is not None and b % 2 == 1:
                            post_main(b // 2)

                # ======================= LAYER 1 =======================
                ycat1 = big.tile([48, NT], F32, tag="cat", name='ycat1')
                ndtb1 = big.tile([128, 2 * B * NS], F32, tag="ndtb",
                                 name='ndtb1')
                xv = xun[0:12, :].rearrange("c (b h w) -> c b h w", b=B, h=16,
                                            w=16)
                h2 = big.tile([65, NT], F32, tag="act", name='h2')
                h2b = big.tile([64, NT], BF16, tag="hb", name='h2b')

                def w1p_chunk(ch):
                    p3 = ps.tile([64, 512], F32, tag="pc", name=f'p1_{ch}')
                    nc.tensor.matmul(p3[:], w1p_l[0:16, :],
                                     ycat1[0:16, ch * 512:(ch + 1) * 512],
                                     start=True, stop=False)
                    nc.tensor.matmul(p3[:], w1p_l[32:48, :],
                                     ycat1[32:48, ch * 512:(ch + 1) * 512],
                                     start=False, stop=True)
                    nc.scalar.activation(h2[0:64, ch * 512:(ch + 1) * 512],
                                         p3[:], AF.Identity, bias=b1p[:])
                    nc.scalar.activation(h2b[:, ch * 512:(ch + 1) * 512],
                                         p3[:], AF.Identity, bias=b1p[:])

                score_loop(1, xun, 13, samp1, ndtb1)
                conv_loop(1, 16, wc1, 12, xv, ycat1, b1a)
                p_loop(1, samp1, 16, w1bkT, 16, Pt1, dup1sc)
                mask_loop(1, ndtb1, 16, Pt1, ycat1, b1b, F32,
                          post_main=w1p_chunk)
                nc.vector.memset(h2[64:65, :], 1.0)
                if stage == 1:
                    nc.sync.dma_start(h2_d[:], h2[0:64, :])
                    nc.sync.dma_start(yc1_d[:], ycat1[:])
                    nc.sync.dma_start(ndtb_d[:], ndtb1[:])
                    nc.sync.dma_start(pt1_d[:], Pt1[:].rearrange("p b c o -> p (b c o)"))

                # ======================= LAYER 2 =======================
                samp2 = small.tile([65, 2048], F32, tag="samp", name='samp2')
                ycat2 = big.tile([64, NT], BF16, tag="cat", name='ycat2')
                h2v = h2b[:].rearrange("c (b h w) -> c b h w", b=B,
                                       h=16, w=16)
                conv_loop(2, 32, wconv2, 64, h2v, ycat2, b2a)
                # gather + sample norms in 8-image blocks (pipelines with w1p)
                for g in range(4):
                    nc.gpsimd.ap_gather(
                        samp2[0:64, g * 512:(g + 1) * 512],
                        h2[0:64, g * 2048:(g + 1) * 2048],
                        idxs2[:, 32 * g:32 * (g + 1)],
                        channels=64, num_elems=2048, d=1, num_idxs=512)
                    sq = work.tile([64, 512], F32, tag="sq", name=f'sq_{g}')
                    nc.vector.tensor_mul(sq[:],
                                         samp2[0:64, g * 512:(g + 1) * 512],
                                         samp2[0:64, g * 512:(g + 1) * 512])
                    pssn = ps.tile([1, 512], F32, tag="pd", name=f'sn_{g}')
                    nc.tensor.matmul(pssn[:], onesc2[:], sq[:],
                                     start=True, stop=True)
                    nc.scalar.mul(samp2[64:65, g * 512:(g + 1) * 512],
                                  pssn[:], -0.5)

                ndtb2 = big.tile([128, 2 * B * NS], F32, tag="ndtb",
                                 name='ndtb2')
                yp2 = big.tile([128, NT], BF16, tag="act", name='yp2')
                yp2v = yp2[:].rearrange("c (pos b) -> c b pos", b=B)

                def w2p_chunk(ch):
                    p6 = ps.tile([128, 512], F32, tag="pc", name=f'p2_{ch}')
                    nc.tensor.matmul(p6[:], w2p_l[:],
                                     ycat2[:, ch * 512:(ch + 1) * 512],
                                     start=True, stop=True)
                    nc.scalar.activation(
                        yp2v[:, ch * 2:ch * 2 + 2, :],
                        p6[:].rearrange("c (b pos) -> c b pos", b=2),
                        AF.Identity, bias=b2p[:])

                score_loop(2, h2, 65, samp2, ndtb2)
                p_loop(2, samp2, 64, w2bkT, 32, Pt2, dup2sc)
                mask_loop(2, ndtb2, 32, Pt2, ycat2, b2b, BF16,
                          post_main=w2p_chunk)
                if stage == 2:
                    nc.sync.dma_start(yp2_d[:], yp2[:])

            # ======================= FC head =======================
            with tc.tile_pool(name="psfc", bufs=1, space="PSUM") as psfc, \
                 tc.tile_pool(name="psf2", bufs=2, space="PSUM") as psf2:
                f0 = psfc.tile([32, 512], F32, tag="fc0", name='f0')
                f1 = psfc.tile([32, 512], F32, tag="fc1", name='f1')
                nc.tensor.matmul(f0[:], ones1[:], fc1b2[:, 0:512],
                                 start=True, stop=False)
                nc.tensor.matmul(f1[:], ones1[:], fc1b2[:, 512:1024],
                                 start=True, stop=False)
                for pos in range(T):
                    wt = fc1w_tiles[pos // 4]
                    q = pos % 4
                    lhs = yp2[:, pos * 32:(pos + 1) * 32]
                    nc.tensor.matmul(f0[:], lhs, wt[:, q * 1024:q * 1024 + 512],
                                     start=False, stop=(pos == T - 1))
                    nc.tensor.matmul(f1[:], lhs,
                                     wt[:, q * 1024 + 512:(q + 1) * 1024],
                                     start=False, stop=(pos == T - 1))
                hfcT = small.tile([32, 1024], F32, name='hfcT')
                nc.scalar.activation(hfcT[:, 0:512], f0[:], AF.Relu)
                nc.scalar.activation(hfcT[:, 512:1024], f1[:], AF.Relu)
                hfc2 = small.tile([128, 8, 32], F32, name='hfc2')
                for r in range(8):
                    ptp = psf2.tile([128, 32], F32, tag="tp", name=f'tp{r}')
                    nc.tensor.transpose(ptp[:], hfcT[:, r * 128:(r + 1) * 128],
                                        ident128[0:32, 0:32])
                    nc.scalar.copy(hfc2[:, r, :], ptp[:])
                p7 = psf2.tile([10, 32], F32, tag="fc2o", name='p7')
                for r in range(8):
                    nc.tensor.matmul(p7[:], fc2w[:, r * 10:(r + 1) * 10],
                                     hfc2[:, r, :], start=(r == 0),
                                     stop=(r == 7))
                yo = small.tile([10, 32], F32, name='yo')
                nc.scalar.activation(yo[:], p7[:], AF.Identity, bias=fc2b[:])
                pt = psf2.tile([32, 10], F32, tag="tr", name='pt')
                nc.tensor.transpose(pt[:], yo[:], ident128[0:10, 0:10])
                yout = small.tile([32, 10], F32, name='yout')
                nc.scalar.copy(yout[:], pt[:])
                nc.sync.dma_start(out_d[:], yout[:])
    nc.compile()
    return nc


_NC = None
_NC_STAGE = None


def get_nc(stage=3):
    global _NC, _NC_STAGE
    if _NC is None or _NC_STAGE != stage:
        _NC = build_bass(stage)
        _NC_STAGE = stage
    return _NC


def kernel(**inputs):
    nc = get_nc(3)
    in_maps = prep_core_maps(inputs)
    res = run_bass_kernel_spmd(nc, in_maps, core_ids=list(range(NCORES)))
    return np.concatenate([res.results[c]['out'] for c in range(NCORES)],
                          axis=0)


# revision 53
# speedup vs baseline: 1.0780x; 1.0780x over previous
"""Trainium2 Bass kernel for nn_B_Conv2d_ConvNN_K_N (retrieval_knn).

Data-parallel over 8 NeuronCores: 32 images/core, weights replicated.

KNN neighbor aggregation is reformulated as a one-hot matmul: per image,
P_k = (W_k @ samp)^T, and a rank-k membership mask A_k[t, n] =
(score[t,n] == kth_max[t]) / count, so y[o,t] = sum_k (P_k^T A_k^T)[o,t].
Count normalization makes duplicated sample columns exact.  This removes
all per-token gathers and the DRAM index staging of the previous version.

Convs use unpadded clipped-tap matmuls (border taps write PSUM subranges).
FC1 keeps activations stationary (LDW [128,32] per position) and streams
the 64MB weight through a deep-prefetch tile ring.
"""
import sys
if '/opt/trn_rl_repo' not in sys.path:
    sys.path.insert(0, '/opt/trn_rl_repo')

import numpy as np
import concourse.bacc as bacc
import concourse.mybir as mybir
from concourse.tile import TileContext
from concourse.bass_utils import run_bass_kernel_spmd

dt = mybir.dt
AF = mybir.ActivationFunctionType
ALU = mybir.AluOpType
NCORES = 8
B = 32            # images per core
T = 256           # tokens per image at conv resolution (16x16)
NT = B * T        # 8192
NS = 64           # random samples per image
K = 9             # nearest neighbors
NFCW = 5          # fc1 weight tiles in flight (8KB free-bytes each)


def _bf16(x):
    import ml_dtypes
    return np.asarray(x, np.float32).astype(ml_dtypes.bfloat16)


def prep_weights(w):
    """Core-independent input tensors (weights, indices)."""
    m = {}

    def samp_idx(idx, groups):
        # image index local to each 8-image gather block
        t = np.zeros((16 * groups, 128), np.int16)
        s = np.arange(128)
        for g in range(groups):
            for p in range(16):
                t[16 * g + p, :] = ((s // 4) % 8) * 256 + idx[16 * (s % 4) + p]
        return t
    m['idxs2'] = samp_idx(np.asarray(w['idx2']), 4)

    w1a = np.asarray(w['w1a'], np.float32)       # (16, 12, 3, 3)
    wc1 = np.zeros((109, 16), np.float32)
    for t, (dh, dw) in enumerate(TAPS1):
        r = 0 if t == 0 else 13 + 12 * (t - 1)
        wc1[r:r + 12, :] = w1a[:, :, dh, dw].T
    m['wc1'] = wc1
    w1b = np.asarray(w['w1b'], np.float32)       # (16, 12, 9)
    w1bkT = np.zeros((16, 144), np.float32)
    for k in range(K):
        w1bkT[0:12, k * 16:(k + 1) * 16] = w1b[:, :, k].T
    m['w1bkT'] = w1bkT
    w1p = np.asarray(w['w1p'], np.float32)[:, :, 0, 0]   # (64, 32)
    w1pl = np.zeros((48, 64), np.float32)
    w1pl[0:16] = w1p[:, 0:16].T
    w1pl[32:48] = w1p[:, 16:32].T
    m['w1p_l'] = w1pl

    w2a = np.asarray(w['w2a'], np.float32)       # (32, 64, 3, 3)
    wc2 = np.zeros((64, 288), np.float32)
    for dh in range(3):
        for dw in range(3):
            tap = dh * 3 + dw
            wc2[:, tap * 32:(tap + 1) * 32] = w2a[:, :, dh, dw].T
    m['wconv2'] = _bf16(wc2)
    w2b = np.asarray(w['w2b'], np.float32)       # (32, 64, 9)
    w2bkT = np.zeros((64, 288), np.float32)
    for k in range(K):
        w2bkT[:, k * 32:(k + 1) * 32] = w2b[:, :, k].T
    m['w2bkT'] = w2bkT
    w2p = np.asarray(w['w2p'], np.float32)[:, :, 0, 0]   # (128, 64)
    m['w2p_l'] = _bf16(np.ascontiguousarray(w2p.T))      # (64, 128)

    m['b1a'] = np.asarray(w['b1a']).reshape(16, 1).astype(np.float32)
    m['b1b'] = np.asarray(w['b1b']).reshape(16, 1).astype(np.float32)
    m['b1p'] = np.asarray(w['b1p']).reshape(64, 1).astype(np.float32)
    m['b2a'] = np.asarray(w['b2a']).reshape(32, 1).astype(np.float32)
    m['b2b'] = np.asarray(w['b2b']).reshape(32, 1).astype(np.float32)
    m['b2p'] = np.asarray(w['b2p']).reshape(128, 1).astype(np.float32)
    m['onesc2'] = np.ones((64, 1), np.float32)
    m['ones1'] = np.ones((1, 32), np.float32)
    m['neghalf'] = np.full((1, 128), -0.5, np.float32)
    def dup_scale(idx):
        idx = np.asarray(idx)
        mult = np.array([(idx == v).sum() for v in idx], np.float32)
        return np.tile(1.0 / mult, 2).reshape(128, 1)
    m['dup1sc'] = dup_scale(w['idx1'])
    m['dup2sc'] = dup_scale(w['idx2'])
    m['ident128'] = np.eye(128, dtype=np.float32)
    m['ident128b'] = _bf16(np.eye(128, dtype=np.float32))

    # FC1 weights per spatial position, bf16, 4-pos tiles (64, 128, 4096)
    fc1 = np.asarray(w['fc1w']).reshape(1024, 32, 32, 32)   # f, cc, hh, ww
    fc1 = fc1.reshape(1024, 32, 16, 2, 16, 2)               # f, cc, h, i, w, j
    fc1 = fc1.transpose(2, 4, 1, 3, 5, 0)                   # h, w, cc, i, j, f
    fc1 = fc1.reshape(256, 128, 1024)                       # pos, ch, f
    m['fc1wp'] = _bf16(fc1.reshape(64, 4, 128, 1024).transpose(0, 2, 1, 3)
                       .reshape(64, 128, 4096))
    m['fc1b2'] = np.asarray(w['fc1b']).reshape(1, 1024).astype(np.float32)
    m['fc2w_l'] = np.ascontiguousarray(np.asarray(w['fc2w']).T).astype(np.float32)
    m['fc2b'] = np.asarray(w['fc2b']).reshape(10, 1).astype(np.float32)
    return m


TAPS1 = [(1, 1)] + [(dh, dw) for dh in range(3) for dw in range(3)
                    if not (dh == 1 and dw == 1)]


def prep_x(x_core, idx1):
    """Per-core tensors: im2col-stacked x (9 taps + ones row),
    host-gathered samples with the -sn/2 row."""
    xu = x_core.reshape(B, 3, 16, 2, 16, 2).transpose(1, 3, 5, 0, 2, 4)
    xu = np.ascontiguousarray(xu).reshape(12, B, 16, 16).astype(np.float32)
    xsh = np.zeros((109, B, 16, 16), np.float32)
    xsh[12] = 1.0
    for t, (dh, dw) in enumerate(TAPS1):
        r = 0 if t == 0 else 13 + 12 * (t - 1)
        hl, hh = max(0, 1 - dh), min(16, 17 - dh)
        wl, wh = max(0, 1 - dw), min(16, 17 - dw)
        xsh[r:r + 12, :, hl:hh, wl:wh] = \
            xu[:, :, hl + dh - 1:hh + dh - 1, wl + dw - 1:wh + dw - 1]
    xsh = xsh.reshape(109, NT)
    samp = xu.reshape(12, B, T)[:, :, idx1]        # (12, B, 64)
    samp1 = np.zeros((16, B * NS), np.float32)
    samp1[:12] = samp.reshape(12, B * NS)
    samp1[12] = -0.5 * (samp1[:12] ** 2).sum(axis=0)
    return {'xsh': xsh, 'samp1': samp1}


def prep_core_maps(inputs):
    x = np.asarray(inputs['x'], np.float32)
    idx1 = np.asarray(inputs['idx1'])
    wmap = prep_weights(inputs)
    maps = []
    for c in range(NCORES):
        m = dict(wmap)
        m.update(prep_x(x[c * B:(c + 1) * B], idx1))
        maps.append(m)
    return maps


def build_bass(stage=3):
    F32, I16, BF16 = dt.float32, dt.int16, dt.bfloat16
    nc = bacc.Bacc("TRN2", target_bir_lowering=False, debug=False)

    def din(name, shape, d=F32):
        return nc.dram_tensor(name, shape, d, kind="ExternalInput")

    xsh_d = din('xsh', [109, NT])
    samp1_d = din('samp1', [16, 2048])
    idxs2_d = din('idxs2', [64, 128], I16)
    wc1_d = din('wc1', [109, 16])
    w1bkT_d = din('w1bkT', [16, 144])
    w1p_l_d = din('w1p_l', [48, 64])
    wconv2_d = din('wconv2', [64, 288], BF16)
    w2bkT_d = din('w2bkT', [64, 288])
    w2p_l_d = din('w2p_l', [64, 128], BF16)
    b1a_d, b1b_d, b1p_d = din('b1a', [16, 1]), din('b1b', [16, 1]), din('b1p', [64, 1])
    b2a_d, b2b_d, b2p_d = din('b2a', [32, 1]), din('b2b', [32, 1]), din('b2p', [128, 1])
    onesc2_d = din('onesc2', [64, 1])
    ones1_d = din('ones1', [1, 32])
    dup1sc_d = din('dup1sc', [128, 1])
    dup2sc_d = din('dup2sc', [128, 1])
    ident128_d = din('ident128', [128, 128])
    ident128b_d = din('ident128b', [128, 128], BF16)
    fc1wp_d = din('fc1wp', [64, 128, 4096], BF16)
    fc1b2_d = din('fc1b2', [1, 1024])
    fc2w_l_d = din('fc2w_l', [1024, 10])
    fc2b_d = din('fc2b', [10, 1])
    out_d = nc.dram_tensor('out', [B, 10], F32, kind="ExternalOutput")
    if stage == 1:
        h2_d = nc.dram_tensor('h2_dbg', [64, NT], F32, kind="ExternalOutput")
        yc1_d = nc.dram_tensor('yc1_dbg', [48, NT], F32, kind="ExternalOutput")
        ndtb_d = nc.dram_tensor('ndtb_dbg', [128, 4096], F32, kind="ExternalOutput")
        pt1_d = nc.dram_tensor('pt1_dbg', [128, B * 5 * 16], F32, kind="ExternalOutput")
        at0_d = nc.dram_tensor('at0_dbg', [128, 5 * 256], F32, kind="ExternalOutput")
    if stage == 2:
        yp2_d = nc.dram_tensor('yp2_dbg', [128, NT], BF16, kind="ExternalOutput")

    with TileContext(nc) as tc:
        with tc.tile_pool(name="consts", bufs=1) as cpool, \
             tc.tile_pool(name="big", bufs=1) as big, \
             tc.tile_pool(name="att", bufs=2) as att, \
             tc.tile_pool(name="work", bufs=3) as work, \
             tc.tile_pool(name="apool", bufs=4) as apool, \
             tc.tile_pool(name="small", bufs=1) as small, \
             tc.tile_pool(name="fcw", bufs=NFCW) as fcw:

            def load(dram_t, name, d=None):
                t = cpool.tile(list(dram_t.shape), d or dram_t.dtype, name=name)
                nc.scalar.dma_start(t[:], dram_t[:])
                return t

            wc1 = load(wc1_d, 'wc1')
            w1bkT = load(w1bkT_d, 'w1bkT')
            w1p_l = load(w1p_l_d, 'w1p_l')
            wconv2 = load(wconv2_d, 'wconv2')
            w2bkT = load(w2bkT_d, 'w2bkT')
            w2p_l = load(w2p_l_d, 'w2p_l')
            b1a, b1b, b1p = load(b1a_d, 'b1a'), load(b1b_d, 'b1b'), load(b1p_d, 'b1p')
            b2a, b2b, b2p = load(b2a_d, 'b2a'), load(b2b_d, 'b2b'), load(b2p_d, 'b2p')
            onesc2 = load(onesc2_d, 'onesc2')
            ones1 = load(ones1_d, 'ones1')
            dup1sc = load(dup1sc_d, 'dup1sc')
            dup2sc = load(dup2sc_d, 'dup2sc')
            ident128 = load(ident128_d, 'ident128')
            ident128b = load(ident128b_d, 'ident128b')
            fc1b2 = load(fc1b2_d, 'fc1b2')
            fc2b = load(fc2b_d, 'fc2b')
            idxs2 = load(idxs2_d, 'idxs2')
            samp1 = small.tile([16, 2048], F32, tag="samp", name='samp1')
            nc.scalar.dma_start(samp1[:], samp1_d[:])
            fc2w = cpool.tile([128, 80], F32, name='fc2w')
            for r in range(8):
                nc.scalar.dma_start(fc2w[:, r * 10:(r + 1) * 10],
                                    fc2w_l_d[r * 128:(r + 1) * 128, :])

            xsh = big.tile([109, NT], F32, tag="act", name='xsh')
            nc.sync.dma_start(xsh[:], xsh_d[:])

            # FC1 weight stream (program-order early; consumed at the end)
            fc1w_tiles = []
            for i in range(64):
                ft = fcw.tile([128, 4096], BF16, tag="fc1w", name=f'fc1w{i}')
                eng = (nc.sync, nc.scalar)[i % 2]
                eng.dma_start(ft[:], fc1wp_d[i])
                fc1w_tiles.append(ft)

            # Per-image P tiles: rows kn (k*64+n) in 5 chunks of 128, cols o
            Pt1 = cpool.tile([128, B, 5, 16], F32, name='Pt1')
            Pt2 = cpool.tile([128, B, 5, 32], BF16, name='Pt2')

            with tc.tile_pool(name="ps", bufs=2, space="PSUM") as ps, \
                 tc.tile_pool(name="pst", bufs=2, space="PSUM") as pst:

                def conv_loop(lay, o_ch, wtap, cin, srcv, ycat, biasc):
                    taps = [(1, 1)] + [(dh, dw) for dh in range(3)
                                       for dw in range(3)
                                       if not (dh == 1 and dw == 1)]
                    for ch in range(16):
                        b0 = ch * 2
                        pc = ps.tile([o_ch, 2, 16, 16], F32, tag="pc",
                                     name=f'c{lay}_{ch}')
                        for i, (dh, dw) in enumerate(taps):
                            tap = dh * 3 + dw
                            hl, hh = max(0, 1 - dh), min(16, 17 - dh)
                            wl, wh = max(0, 1 - dw), min(16, 17 - dw)
                            nc.tensor.matmul(
                                pc[:, :, hl:hh, wl:wh],
                                wtap[:, tap * o_ch:(tap + 1) * o_ch],
                                srcv[0:cin, b0:b0 + 2,
                                     hl + dh - 1:hh + dh - 1,
                                     wl + dw - 1:wh + dw - 1],
                                start=(i == 0), stop=(i == 8))
                        nc.scalar.activation(
                            ycat[0:o_ch, ch * 512:(ch + 1) * 512],
                            pc[:].rearrange("o b h w -> o (b h w)"),
                            AF.Relu, bias=biasc[:])

                def score_loop(lay, srcT, ctr_s, samp, ndtb):
                    # row ctr_s-1 of srcT is 1.0 and of samp is -sn/2, so a
                    # single matmul yields dots - sn/2
                    for i in range(B):
                        pd = ps.tile([128, 128], F32, tag="pd",
                                     name=f'd{lay}_{i}')
                        for h in range(2):
                            bh = 2 * i + h
                            nc.tensor.matmul(
                                pd[:, h * NS:(h + 1) * NS],
                                srcT[0:ctr_s, bh * 128:(bh + 1) * 128],
                                samp[0:ctr_s, i * NS:(i + 1) * NS],
                                start=True, stop=True)
                        nc.scalar.copy(ndtb[:, i * 128:(i + 1) * 128], pd[:])

                def p_loop(lay, samp, ctr_p, wbkT, o_ch, Pt, dupsc):
                    for b in range(B):
                        pP = ps.tile([128, 5, o_ch], F32, tag="pP",
                                     name=f'pP{lay}_{b}')
                        for c in range(5):
                            for kk in range(1 if c == 4 else 2):
                                k = 2 * c + kk
                                nc.tensor.matmul(
                                    pP[64 * kk:64 * kk + 64, c, :],
                                    samp[0:ctr_p, b * NS:(b + 1) * NS],
                                    wbkT[0:ctr_p, k * o_ch:(k + 1) * o_ch],
                                    start=True, stop=True)
                        nc.scalar.mul(Pt[:, b, :, :], pP[:], dupsc[:])

                def mask_loop(lay, ndtb, o_ch, Pt, ycat, biasb, adt,
                              post_main=None):
                    """Rank one-hot masks -> transposed one-hot matmul."""
                    for b in range(B):
                        AT = att.tile([128, 5, 256], adt, tag="AT",
                                      name=f'AT{lay}_{b}')
                        As = []
                        for h in range(2):
                            bh = b * 2 + h
                            ndt = ndtb[:, bh * NS:(bh + 1) * NS]
                            mxc = work.tile([128, 9], F32, tag="mxc",
                                            name=f'mxc{lay}_{bh}')
                            nc.vector.max(mxc[:, 0:8], ndt)
                            nd2 = work.tile([128, NS], F32, tag="nd2",
                                            name=f'n2_{lay}_{bh}')
                            nc.vector.match_replace(nd2[:], mxc[:, 0:8], ndt,
                                                    -1e30)
                            t9 = work.tile([128, 8], F32, tag="t9",
                                           name=f't9_{lay}_{bh}')
                            nc.vector.max(t9[:], nd2[:])
                            nc.vector.tensor_copy(mxc[:, 8:9], t9[:, 0:1])
                            A = apool.tile([128, 576], adt, tag="A",
                                           name=f'A{lay}_{bh}')
                            nc.vector.tensor_tensor(
                                A[:].rearrange("p (k n) -> p k n", n=NS),
                                ndt.unsqueeze(1).broadcast_to([128, K, NS]),
                                mxc[:].unsqueeze(2).broadcast_to([128, K, NS]),
                                ALU.is_equal)
                            As.append(A)
                        # transpose both halves of chunk c into one PSUM tile,
                        # evacuate with a single scalar copy
                        for c in range(5):
                            W = 64 if c == 4 else 128
                            ptr = pst.tile([128, 256], adt, tag="ptr",
                                           name=f'tr{lay}_{b}_{c}')
                            ident = ident128 if adt == F32 else ident128b
                            for h in range(2):
                                nc.tensor.transpose(
                                    ptr[0:W, h * 128:(h + 1) * 128],
                                    As[h][:, c * 128:c * 128 + W],
                                    ident[:])
                            nc.scalar.copy(AT[0:W, c, :], ptr[0:W, :])
                        if stage == 1 and lay == 1 and b == 0:
                            nc.sync.dma_start(at0_d[:], AT[:])
                        pm = ps.tile([o_ch, 256], F32, tag="pd",
                                     name=f'pm{lay}_{b}')
                        for c in range(5):
                            W = 64 if c == 4 else 128
                            nc.tensor.matmul(pm[:], Pt[0:W, b, c, :],
                                             AT[0:W, c, :],
                                             start=(c == 0), stop=(c == 4))
                        nc.scalar.activation(
                            ycat[32:32 + o_ch, b * T:(b + 1) * T], pm[:],
                            AF.Relu, bias=biasb[:])
                        if post_main is not None and b % 2 == 1:
                            post_main(b // 2)

                # ======================= LAYER 1 =======================
                ycat1 = big.tile([48, NT], F32, tag="cat", name='ycat1')
                ndtb1 = big.tile([128, 2 * B * NS], F32, tag="ndtb",
                                 name='ndtb1')
                h2 = big.tile([65, NT], F32, tag="act", name='h2')
                h2b = big.tile([64, NT], BF16, tag="hb", name='h2b')

                def w1p_chunk(ch):
                    p3 = ps.tile([64, 512], F32, tag="pc", name=f'p1_{ch}')
                    nc.tensor.matmul(p3[:], w1p_l[0:16, :],
                                     ycat1[0:16, ch * 512:(ch + 1) * 512],
                                     start=True, stop=False)
                    nc.tensor.matmul(p3[:], w1p_l[32:48, :],
                                     ycat1[32:48, ch * 512:(ch + 1) * 512],
                                     start=False, stop=True)
                    nc.scalar.activation(h2[0:64, ch * 512:(ch + 1) * 512],
                                         p3[:], AF.Identity, bias=b1p[:])
                    nc.scalar.activation(h2b[:, ch * 512:(ch + 1) * 512],
                                         p3[:], AF.Identity, bias=b1p[:])

                score_loop(1, xsh, 13, samp1, ndtb1)
                for ch in range(16):
                    pcc = ps.tile([16, 512], F32, tag="pc", name=f'c1_{ch}')
                    nc.tensor.matmul(pcc[:], wc1[:],
                                     xsh[:, ch * 512:(ch + 1) * 512],
                                     start=True, stop=True)
                    nc.scalar.activation(ycat1[0:16, ch * 512:(ch + 1) * 512],
                                         pcc[:], AF.Relu, bias=b1a[:])
                p_loop(1, samp1, 16, w1bkT, 16, Pt1, dup1sc)
                mask_loop(1, ndtb1, 16, Pt1, ycat1, b1b, F32,
                          post_main=w1p_chunk)
                nc.vector.memset(h2[64:65, :], 1.0)
                if stage == 1:
                    nc.sync.dma_start(h2_d[:], h2[0:64, :])
                    nc.sync.dma_start(yc1_d[:], ycat1[:])
                    nc.sync.dma_start(ndtb_d[:], ndtb1[:])
                    nc.sync.dma_start(pt1_d[:], Pt1[:].rearrange("p b c o -> p (b c o)"))

                # ======================= LAYER 2 =======================
                samp2 = small.tile([65, 2048], F32, tag="samp", name='samp2')
                ycat2 = big.tile([64, NT], BF16, tag="cat", name='ycat2')
                h2v = h2b[:].rearrange("c (b h w) -> c b h w", b=B,
                                       h=16, w=16)
                conv_loop(2, 32, wconv2, 64, h2v, ycat2, b2a)
                # gather + sample norms in 8-image blocks (pipelines with w1p)
                for g in range(4):
                    nc.gpsimd.ap_gather(
                        samp2[0:64, g * 512:(g + 1) * 512],
                        h2[0:64, g * 2048:(g + 1) * 2048],
                        idxs2[:, 32 * g:32 * (g + 1)],
                        channels=64, num_elems=2048, d=1, num_idxs=512)
                    sq = work.tile([64, 512], F32, tag="sq", name=f'sq_{g}')
                    nc.vector.tensor_mul(sq[:],
                                         samp2[0:64, g * 512:(g + 1) * 512],
                                         samp2[0:64, g * 512:(g + 1) * 512])
                    pssn = ps.tile([1, 512], F32, tag="pd", name=f'sn_{g}')
                    nc.tensor.matmul(pssn[:], onesc2[:], sq[:],
                                     start=True, stop=True)
                    nc.scalar.mul(samp2[64:65, g * 512:(g + 1) * 512],
                                  pssn[:], -0.5)

                ndtb2 = big.tile([128, 2 * B * NS], F32, tag="ndtb",
                                 name='ndtb2')
                yp2 = big.tile([128, NT], BF16, tag="act", name='yp2')
                yp2v = yp2[:].rearrange("c (pos b) -> c b pos", b=B)

                def w2p_chunk(ch):
                    p6 = ps.tile([128, 512], F32, tag="pc", name=f'p2_{ch}')
                    nc.tensor.matmul(p6[:], w2p_l[:],
                                     ycat2[:, ch * 512:(ch + 1) * 512],
                                     start=True, stop=True)
                    nc.scalar.activation(
                        yp2v[:, ch * 2:ch * 2 + 2, :],
                        p6[:].rearrange("c (b pos) -> c b pos", b=2),
                        AF.Identity, bias=b2p[:])

                score_loop(2, h2, 65, samp2, ndtb2)
                p_loop(2, samp2, 64, w2bkT, 32, Pt2, dup2sc)
                mask_loop(2, ndtb2, 32, Pt2, ycat2, b2b, BF16,
                          post_main=w2p_chunk)
                if stage == 2:
                    nc.sync.dma_start(yp2_d[:], yp2[:])

            # ======================= FC head =======================
            with tc.tile_pool(name="psfc", bufs=1, space="PSUM") as psfc, \
                 tc.tile_pool(name="psf2", bufs=2, space="PSUM") as psf2:
                fb = [psfc.tile([32, 512], F32, tag=f"fc{j}", name=f'f{j}')
                      for j in range(4)]
                nc.tensor.matmul(fb[0][:], ones1[:], fc1b2[:, 0:512],
                                 start=True, stop=False)
                nc.tensor.matmul(fb[1][:], ones1[:], fc1b2[:, 512:1024],
                                 start=True, stop=False)
                for pos in range(T):
                    wt = fc1w_tiles[pos // 4]
                    q = pos % 4
                    lhs = yp2[:, pos * 32:(pos + 1) * 32]
                    j = 2 * (pos % 2)
                    nc.tensor.matmul(fb[j][:], lhs,
                                     wt[:, q * 1024:q * 1024 + 512],
                                     start=(pos == 1), stop=(pos >= T - 2))
                    nc.tensor.matmul(fb[j + 1][:], lhs,
                                     wt[:, q * 1024 + 512:(q + 1) * 1024],
                                     start=(pos == 1), stop=(pos >= T - 2))
                hfcp = small.tile([32, 1024], F32, name='hfcp')
                nc.scalar.copy(hfcp[:, 0:512], fb[0][:])
                nc.scalar.copy(hfcp[:, 512:1024], fb[1][:])
                nc.vector.tensor_add(hfcp[:, 0:512], hfcp[:, 0:512], fb[2][:])
                nc.vector.tensor_add(hfcp[:, 512:1024], hfcp[:, 512:1024],
                                     fb[3][:])
                hfcT = small.tile([32, 1024], F32, name='hfcT')
                nc.scalar.activation(hfcT[:], hfcp[:], AF.Relu)
                hfc2 = small.tile([128, 8, 32], F32, name='hfc2')
                for r in range(8):
                    ptp = psf2.tile([128, 32], F32, tag="tp", name=f'tp{r}')
                    nc.tensor.transpose(ptp[:], hfcT[:, r * 128:(r + 1) * 128],
                                        ident128[0:32, 0:32])
                    nc.scalar.copy(hfc2[:, r, :], ptp[:])
                p7 = psf2.tile([10, 32], F32, tag="tp", name='p7')
                for r in range(8):
                    nc.tensor.matmul(p7[:], fc2w[:, r * 10:(r + 1) * 10],
                                     hfc2[:, r, :], start=(r == 0),
                                     stop=(r == 7))
                yo = small.tile([10, 32], F32, name='yo')
                nc.scalar.activation(yo[:], p7[:], AF.Identity, bias=fc2b[:])
                pt = psf2.tile([32, 10], F32, tag="tp", name='pt')
                nc.tensor.transpose(pt[:], yo[:], ident128[0:10, 0:10])
                yout = small.tile([32, 10], F32, name='yout')
                nc.scalar.copy(yout[:], pt[:])
                nc.sync.dma_start(out_d[:], yout[:])
    nc.compile()
    return nc


_NC = None
_NC_STAGE = None


def get_nc(stage=3):
    global _NC, _NC_STAGE
    if _NC is None or _NC_STAGE != stage:
        _NC = build_bass(stage)
        _NC_STAGE = stage
    return _NC


def kernel(**inputs):
    nc = get_nc(3)
    in_maps = prep_core_maps(inputs)
    res = run_bass_kernel_spmd(nc, in_maps, core_ids=list(range(NCORES)))
    return np.concatenate([res.results[c]['out'] for c in range(NCORES)],
                          axis=0)
